# revision 13
# baseline (speedup 1.0000x reference)
"""Trainium2 Bass kernel for nn_CrossAttention (B=2, S=2048, E=1024, H=16, d=64).

Sharding: 8 cores = 2 batches x 4 query-blocks of 512 rows. Each core gets its
query block + the full values[b] for its batch; no collectives needed.

Algebra (host-folded): with q = q_in@Wq.T+bq, v = v_in@Wv.T+bv, k = v@Wk.T+bk:
  scores = q @ k.T = qe @ v_in.T + const(q-row)   [softmax-shift-invariant]
    where qe = q_in @ (Wq.T@Wk@Wv) + bq@Wk@Wv
  out    = attn @ v @ Wd.T + bd = (attn @ v_in) @ (Wd@blockdiag(Wv)).T
           + (bd + Wd@tile(bv))                   [attn rows sum to 1]
So the device never computes k or v projections.

Per-core device pipeline (feature-major / transposed layouts):
  1. qeT = WQ2 @ q_inT + cq2 per head-pair (WQ2 = blkdiag pair of fold)
  2. scores.T = v_inT.T-slices @ qeT   (row-tiled 2-head concurrent matmuls)
  3. E = exp(scores.T * 0.125): split between ACT (exact spline) and DVE
     (Schraudolph: bf16-bits = int16(x*A + B), ~3.3% max rel err)
  4. U.T = [v_in | ones].T @ E  (ones col yields softmax denom as row 64)
  5. out.T = U.T * (1/r) via PE-replicated reciprocal rows
  6. final = out.T-slices @ WdT' + bd'  -> natural layout -> DMA
"""

import sys

for _p in ("/opt/trn_rl_repo",):
    if _p not in sys.path:
        sys.path.insert(0, _p)

from contextlib import ExitStack

import ml_dtypes
import numpy as np

import concourse.bass as bass
import concourse.tile as tile
from concourse import bacc, mybir
from concourse.bass_utils import run_bass_kernel_spmd

F32 = mybir.dt.float32
BF16 = mybir.dt.bfloat16
F32R = mybir.dt.float32r
I16 = mybir.dt.int16
EXP = mybir.ActivationFunctionType.Exp
MULT = mybir.AluOpType.mult
ADD = mybir.AluOpType.add

B, S, E, H, D = 2, 2048, 1024, 16, 64
N_CORES = 8
SQB = S * B // N_CORES  # 512 query rows per core
NP_BF16 = ml_dtypes.bfloat16

# Schraudolph fast-exp constants: bf16bits(exp(x/8)) ~= int16(x*SCH_A + SCH_B)
LOG2E = 1.4426950408889634
SCH_A = 0.125 * 128.0 * LOG2E
SCH_B = 16256.0 - 5.5027  # HW rounds-to-nearest (probe-verified)

# per pair: psB tiles with g < ACT_B go to ACT, rest to DVE Schraudolph
ACT_B = 2

_CACHE = {}


def _build_program():
    nc = bacc.Bacc("TRN2", target_bir_lowering=False, debug=False, num_devices=N_CORES)

    qT_in = nc.dram_tensor("qT_in", [E, SQB], BF16, kind="ExternalInput").ap()
    vT_in = nc.dram_tensor("vT_in", [E, S], BF16, kind="ExternalInput").ap()
    vnat_in = nc.dram_tensor("vnat_in", [S, H * 65], BF16, kind="ExternalInput").ap()
    wq2 = nc.dram_tensor("wq2", [128, 128], BF16, kind="ExternalInput").ap()
    cq2 = nc.dram_tensor("cq2", [128, 1], F32, kind="ExternalInput").ap()
    sel = nc.dram_tensor("sel", [2, 128], F32R, kind="ExternalInput").ap()
    wdT = nc.dram_tensor("wdT", [E, E], BF16, kind="ExternalInput").ap()
    bd_rep = nc.dram_tensor("bd_rep", [128, E], F32, kind="ExternalInput").ap()
    out = nc.dram_tensor("out", [SQB, E], F32, kind="ExternalOutput").ap()

    with tile.TileContext(nc) as tc, ExitStack() as ctx:
        # ---- pools ----
        wpool = ctx.enter_context(tc.tile_pool(name="w", bufs=1))
        ep = ctx.enter_context(tc.tile_pool(name="ep", bufs=24))
        u2p = ctx.enter_context(tc.tile_pool(name="u2", bufs=1))
        outp = ctx.enter_context(tc.tile_pool(name="outp", bufs=1))
        osbp = ctx.enter_context(tc.tile_pool(name="osb", bufs=2))
        sc_ps = ctx.enter_context(tc.tile_pool(name="scps", bufs=3, space="PSUM"))
        u_ps = ctx.enter_context(tc.tile_pool(name="ups", bufs=2, space="PSUM"))

        # ---- constants / persistent inputs ----
        # DMA order = need order: qe inputs, then per-pair v data, then
        # normalize/output-projection constants (needed only at the tail)
        wq2_s = wpool.tile([128, 128], BF16, tag="wq2")
        nc.sync.dma_start(wq2_s[:], wq2[:])
        cq2_s = wpool.tile([128, 1], F32, tag="cq2")
        nc.sync.dma_start(cq2_s[:], cq2[:])
        qin = []
        for p in range(8):
            t = wpool.tile([128, SQB], BF16, tag=f"qin{p}")
            nc.sync.dma_start(t[:], qT_in[p * 128 : (p + 1) * 128, :])
            qin.append(t)
        # feature-major values, one persistent tile per head-pair
        vt = []
        for p in range(8):
            t = wpool.tile([128, S], BF16, tag=f"vt{p}")
            nc.sync.dma_start(t[:], vT_in[p * 128 : (p + 1) * 128, :])
            vt.append(t)
        # natural values + ones col, one tile per 128-row skv chunk
        vna = []
        for tch in range(16):
            t = wpool.tile([128, H * 65], BF16, tag=f"vna{tch}")
            nc.sync.dma_start(t[:], vnat_in[tch * 128 : (tch + 1) * 128, :])
            vna.append(t)
        sel_s = wpool.tile([2, 128], F32R, tag="sel")
        nc.sync.dma_start(sel_s[:], sel[:])
        bd_s = wpool.tile([128, E], F32, tag="bd")
        nc.sync.dma_start(bd_s[:], bd_rep[:])
        wd_s = []
        for kk in range(8):
            t = wpool.tile([128, E], BF16, tag=f"wd{kk}")
            nc.sync.dma_start(t[:], wdT[kk * 128 : (kk + 1) * 128, :])
            wd_s.append(t)

        # ---- qe projection: all pairs up-front (2 pairs per PSUM tile) ----
        qe_sb = wpool.tile([128, 8 * SQB], BF16, tag="qe")
        for pp in range(4):
            ps = sc_ps.tile([128, 1024], F32, tag="scps")
            for i in range(2):
                p = 2 * pp + i
                nc.tensor.matmul(
                    ps[:, i * 512 : (i + 1) * 512], wq2_s[:], qin[p][:],
                    start=True, stop=True,
                )
            nc.vector.tensor_scalar(
                qe_sb[:, pp * 1024 : (pp + 1) * 1024], ps[:], cq2_s[:], None,
                op0=ADD,
            )

        U2 = [
            u2p.tile([128, SQB], BF16, tag=f"u2_{p}", name=f"u2_{p}")
            for p in range(8)
        ]
        outT = [None] * 8
        oproj_ps = [None] * 4

        # ---- software-pipelined: slot p emits scores(p) interleaved with
        # U(p-1) so the in-order PE stream never blocks on exp output; the
        # per-pair normalize runs inline, and output-projection m-blocks 0/1
        # accumulate during the final slot to shorten the tail ----
        Eprev = None  # (EA, EB) of pair p-1
        for p in range(9):
            qe = qe_sb[:, p * SQB : (p + 1) * SQB] if p < 8 else None
            EA, EB = [], []
            if p > 0:
                upsA = u_ps.tile([65, 512], F32, tag="ups", name=f"upsA{p-1}")
                upsB = u_ps.tile([65, 512], F32, tag="ups", name=f"upsB{p-1}")
            if p == 8:
                oproj_ps[0] = sc_ps.tile([128, 1024], F32, tag="scps", name="oproj0")
                oproj_ps[1] = sc_ps.tile([128, 1024], F32, tag="scps", name="oproj1")
            for g in range(8):
                if p < 8:
                    psA = sc_ps.tile([128, 1024], F32, tag="scps")
                    psB = sc_ps.tile([128, 1024], F32, tag="scps")
                    for tt in range(2):
                        t = g * 2 + tt
                        nc.tensor.matmul(
                            psA[:, tt * 512 : (tt + 1) * 512],
                            vt[p][0:64, t * 128 : (t + 1) * 128],
                            qe[0:64, :],
                            start=True, stop=True, tile_position=(0, 0),
                        )
                        nc.tensor.matmul(
                            psB[:, tt * 512 : (tt + 1) * 512],
                            vt[p][64:128, t * 128 : (t + 1) * 128],
                            qe[64:128, :],
                            start=True, stop=True, tile_position=(64, 0),
                        )
                if p > 0:
                    # U(p-1): chunks 2g, 2g+1 for both heads of the pair
                    pEA, pEB = Eprev
                    for h2, ups, EEp in ((0, upsA, pEA), (1, upsB, pEB)):
                        h = 2 * (p - 1) + h2
                        for t in (2 * g, 2 * g + 1):
                            et = EEp[t // 2][:, (t % 2) * 512 : (t % 2 + 1) * 512]
                            nc.tensor.matmul(
                                ups[:], vna[t][:, h * 65 : (h + 1) * 65], et,
                                start=(t == 0), stop=(t == 15),
                            )
                if p == 8 and g < 7:
                    # output projection m-blocks 0/1, contraction chunk kk=g
                    # (kk=7 needs outT[7], finished only after this slot)
                    for m in range(2):
                        for n in range(2):
                            nc.tensor.matmul(
                                oproj_ps[m][:, n * 512 : (n + 1) * 512],
                                outT[g][:, m * 128 : (m + 1) * 128],
                                wd_s[g][:, n * 512 : (n + 1) * 512],
                                start=(g == 0), stop=False,
                                skip_group_check=True,
                            )
                if p < 8:
                    ea = ep.tile([128, 1024], BF16, tag="E", name=f"ea{p}_{g}")
                    nc.scalar.activation(ea[:], psA[:], EXP, scale=0.125)
                    EA.append(ea)
                    eb = ep.tile([128, 1024], BF16, tag="E", name=f"eb{p}_{g}")
                    if g < ACT_B:
                        nc.scalar.activation(eb[:], psB[:], EXP, scale=0.125)
                    else:
                        nc.vector.tensor_scalar(
                            eb[:].bitcast(I16), psB[:], SCH_A, SCH_B,
                            op0=MULT, op1=ADD,
                        )
                    EB.append(eb)
            if p > 0:
                pm1 = p - 1
                # evacuate U(p-1); row 64 = softmax denominator
                rg2 = osbp.tile([2, SQB], F32, tag="rg2", name=f"rg2_{pm1}")
                for h2, ups in ((0, upsA), (1, upsB)):
                    nc.vector.tensor_copy(
                        U2[pm1][h2 * 64 : (h2 + 1) * 64, :], ups[0:64, :]
                    )
                    # single-partition writes at nonzero offsets are illegal
                    # for compute engines (and DMA can't read PSUM): h0's denom
                    # row lands at partition 0 directly; h1's bounces via DMA
                    if h2 == 0:
                        nc.scalar.copy(rg2[0:1, :], ups[64:65, :])
                    else:
                        rtmp = osbp.tile(
                            [1, SQB], F32, tag="rtmp", name=f"rtmp{pm1}"
                        )
                        nc.vector.tensor_copy(rtmp[:], ups[64:65, :])
                        nc.sync.dma_start(rg2[1:2, :], rtmp[:])
                # inline normalize of pair p-1
                rr2 = osbp.tile([2, SQB], F32R, tag="rr2", name=f"rr2_{pm1}")
                with nc.allow_low_precision(reason="f32r full fp32 range; f22 mantissa ok for softmax denom"):
                    nc.vector.reciprocal(rr2[:], rg2[:])
                rps = sc_ps.tile([128, 1024], F32, tag="scps")
                nc.tensor.matmul(
                    rps[:, 0:512], sel_s[:], rr2[:],
                    start=True, stop=True,
                )
                ot = outp.tile([128, SQB], BF16, tag=f"outT{pm1}")
                nc.vector.tensor_mul(ot[:], U2[pm1][:], rps[:, 0:512])
                outT[pm1] = ot
            Eprev = (EA, EB)

        # ---- finish output projection: kk=7 for m=0/1, all kk for m=2/3 ----
        for m in range(2):
            for n in range(2):
                nc.tensor.matmul(
                    oproj_ps[m][:, n * 512 : (n + 1) * 512],
                    outT[7][:, m * 128 : (m + 1) * 128],
                    wd_s[7][:, n * 512 : (n + 1) * 512],
                    start=False, stop=True, skip_group_check=True,
                )
            osb = osbp.tile([128, E], F32, tag="osb")
            nc.vector.tensor_add(osb[:], oproj_ps[m][:], bd_s[:])
            nc.sync.dma_start(out[m * 128 : (m + 1) * 128, :], osb[:])
        for m in range(2, 4):
            oproj_ps[m] = sc_ps.tile([128, 1024], F32, tag="scps", name=f"oproj{m}")
            for n in range(2):
                for kk in range(8):
                    nc.tensor.matmul(
                        oproj_ps[m][:, n * 512 : (n + 1) * 512],
                        outT[kk][:, m * 128 : (m + 1) * 128],
                        wd_s[kk][:, n * 512 : (n + 1) * 512],
                        start=(kk == 0), stop=(kk == 7),
                    )
            osb = osbp.tile([128, E], F32, tag="osb")
            nc.vector.tensor_add(osb[:], oproj_ps[m][:], bd_s[:])
            nc.sync.dma_start(out[m * 128 : (m + 1) * 128, :], osb[:])

    nc.compile()
    return nc


def kernel(queries, values, heads, Wv, bv, Wk, bk, Wq, bq, Wd, bd, **_):
    queries = np.asarray(queries, np.float32)
    values = np.asarray(values, np.float32)
    Wv, bv = np.asarray(Wv, np.float64), np.asarray(bv, np.float64)
    Wk = np.asarray(Wk, np.float64)
    Wq, bq = np.asarray(Wq, np.float64), np.asarray(bq, np.float64)
    Wd, bd = np.asarray(Wd, np.float64), np.asarray(bd, np.float64)
    assert int(heads) == H and queries.shape == (B, S, E)

    if "nc" not in _CACHE:
        _CACHE["nc"] = _build_program()
    nc = _CACHE["nc"]

    def blk(A):
        Z = np.zeros_like(A)
        return np.block([[A, Z], [Z, A]]).astype(NP_BF16)

    Wkv = Wk @ Wv
    wq2 = blk(Wq.T @ Wkv)                       # lhsT for qe projection
    cq2 = np.tile(Wkv.T @ bq, 2)[:, None].astype(np.float32)
    sel = np.zeros((2, 128), np.float32)
    sel[0, 0:64] = 1.0
    sel[1, 64:128] = 1.0
    bv_full = np.tile(bv, H)
    bd_rep = np.tile((bd + Wd @ bv_full)[None, :], (128, 1)).astype(np.float32)
    Vblk = np.zeros((E, E))
    for h in range(H):
        Vblk[h * D : (h + 1) * D, h * D : (h + 1) * D] = Wv
    wdT = np.ascontiguousarray((Wd @ Vblk).T).astype(NP_BF16)

    vT_b, vnat_b = [], []
    for b_ in range(B):
        vT_b.append(np.ascontiguousarray(values[b_].T).astype(NP_BF16))
        vn = np.empty((S, H * 65), np.float32)
        vr = values[b_].reshape(S, H, D)
        for h in range(H):
            vn[:, h * 65 : h * 65 + 64] = vr[:, h, :]
            vn[:, h * 65 + 64] = 1.0
        vnat_b.append(vn.astype(NP_BF16))

    common = dict(wq2=wq2, cq2=cq2, sel=sel, wdT=wdT, bd_rep=bd_rep)
    in_maps = []
    for c in range(N_CORES):
        b_, qb = c // 4, c % 4
        in_maps.append(dict(
            qT_in=np.ascontiguousarray(
                queries[b_, qb * SQB : (qb + 1) * SQB, :].T
            ).astype(NP_BF16),
            vT_in=vT_b[b_],
            vnat_in=vnat_b[b_],
            **common,
        ))

    _CACHE["last_in_maps"] = in_maps
    res = run_bass_kernel_spmd(nc, in_maps, list(range(N_CORES)))
    out = np.empty((B, S, E), np.float32)
    for c in range(N_CORES):
        b_, qb = c // 4, c % 4
        out[b_, qb * SQB : (qb + 1) * SQB, :] = res.results[c]["out"]
    return out


# revision 14
# speedup vs baseline: 1.0857x; 1.0857x over previous
"""Trainium2 Bass kernel for nn_CrossAttention (B=2, S=2048, E=1024, H=16, d=64).

Sharding: 8 cores = 2 batches x 4 query-blocks of 512 rows. Each core gets its
query block + the full values[b] for its batch; no collectives needed.

Algebra (host-folded): with q = q_in@Wq.T+bq, v = v_in@Wv.T+bv, k = v@Wk.T+bk:
  scores = q @ k.T = qe @ v_in.T + const(q-row)   [softmax-shift-invariant]
    where qe = q_in @ (Wq.T@Wk@Wv) + bq@Wk@Wv
  out    = attn @ v @ Wd.T + bd = (attn @ v_in) @ (Wd@blockdiag(Wv)).T
           + (bd + Wd@tile(bv))                   [attn rows sum to 1]
So the device never computes k or v projections.

Per-core device pipeline (feature-major / transposed layouts):
  1. qeT = WQ2 @ q_inT + cq2 per head-pair (WQ2 = blkdiag pair of fold)
  2. scores.T = v_inT.T-slices @ qeT   (row-tiled 2-head concurrent matmuls)
  3. E = exp(scores.T * 0.125): split between ACT (exact spline) and DVE
     (Schraudolph: bf16-bits = int16(x*A + B), ~3.3% max rel err)
  4. U.T = [v_in | ones].T @ E  (ones col yields softmax denom as row 64)
  5. out.T = U.T * (1/r) via PE-replicated reciprocal rows
  6. final = out.T-slices @ WdT' + bd'  -> natural layout -> DMA
"""

import sys

for _p in ("/opt/trn_rl_repo",):
    if _p not in sys.path:
        sys.path.insert(0, _p)

from contextlib import ExitStack

import ml_dtypes
import numpy as np

import concourse.bass as bass
import concourse.tile as tile
from concourse import bacc, mybir
from concourse.bass_utils import run_bass_kernel_spmd

F32 = mybir.dt.float32
BF16 = mybir.dt.bfloat16
F32R = mybir.dt.float32r
I16 = mybir.dt.int16
EXP = mybir.ActivationFunctionType.Exp
MULT = mybir.AluOpType.mult
ADD = mybir.AluOpType.add

B, S, E, H, D = 2, 2048, 1024, 16, 64
N_CORES = 8
SQB = S * B // N_CORES  # 512 query rows per core
NP_BF16 = ml_dtypes.bfloat16

# Schraudolph fast-exp constants: bf16bits(exp(x/8)) ~= int16(x*SCH_A + SCH_B)
LOG2E = 1.4426950408889634
SCH_A = 0.125 * 128.0 * LOG2E
SCH_B = 16256.0 - 5.5027  # HW rounds-to-nearest (probe-verified)

# per pair: psB tiles with g < ACT_B go to ACT, rest to DVE Schraudolph
ACT_B = 2

_CACHE = {}


def _build_program():
    nc = bacc.Bacc("TRN2", target_bir_lowering=False, debug=False, num_devices=N_CORES)

    qT_in = nc.dram_tensor("qT_in", [E, SQB], BF16, kind="ExternalInput").ap()
    vT_in = nc.dram_tensor("vT_in", [E, S], BF16, kind="ExternalInput").ap()
    vnat_in = nc.dram_tensor("vnat_in", [S, H * 65], BF16, kind="ExternalInput").ap()
    wq2 = nc.dram_tensor("wq2", [128, 128], BF16, kind="ExternalInput").ap()
    cq2 = nc.dram_tensor("cq2", [128, 1], F32, kind="ExternalInput").ap()
    sel = nc.dram_tensor("sel", [2, 128], F32R, kind="ExternalInput").ap()
    wdT = nc.dram_tensor("wdT", [E, E], BF16, kind="ExternalInput").ap()
    bd_rep = nc.dram_tensor("bd_rep", [128, E], F32, kind="ExternalInput").ap()
    out = nc.dram_tensor("out", [SQB, E], F32, kind="ExternalOutput").ap()

    with tile.TileContext(nc) as tc, ExitStack() as ctx:
        # ---- pools ----
        wpool = ctx.enter_context(tc.tile_pool(name="w", bufs=1))
        ep = ctx.enter_context(tc.tile_pool(name="ep", bufs=24))
        u2p = ctx.enter_context(tc.tile_pool(name="u2", bufs=1))
        outp = ctx.enter_context(tc.tile_pool(name="outp", bufs=1))
        osbp = ctx.enter_context(tc.tile_pool(name="osb", bufs=2))
        sc_ps = ctx.enter_context(tc.tile_pool(name="scps", bufs=3, space="PSUM"))
        u_ps = ctx.enter_context(tc.tile_pool(name="ups", bufs=2, space="PSUM"))

        # ---- constants / persistent inputs ----
        # DMA order = need order: qe inputs, then per-pair v data, then
        # normalize/output-projection constants (needed only at the tail)
        wq2_s = wpool.tile([128, 128], BF16, tag="wq2")
        nc.sync.dma_start(wq2_s[:], wq2[:])
        cq2_s = wpool.tile([128, 1], F32, tag="cq2")
        nc.sync.dma_start(cq2_s[:], cq2[:])
        qin = []
        for p in range(8):
            t = wpool.tile([128, SQB], BF16, tag=f"qin{p}")
            nc.sync.dma_start(t[:], qT_in[p * 128 : (p + 1) * 128, :])
            qin.append(t)
        # feature-major values, one persistent tile per head-pair
        vt = []
        for p in range(8):
            t = wpool.tile([128, S], BF16, tag=f"vt{p}")
            nc.sync.dma_start(t[:], vT_in[p * 128 : (p + 1) * 128, :])
            vt.append(t)
        # natural values + ones col, one tile per 128-row skv chunk
        vna = []
        for tch in range(16):
            t = wpool.tile([128, H * 65], BF16, tag=f"vna{tch}")
            nc.sync.dma_start(t[:], vnat_in[tch * 128 : (tch + 1) * 128, :])
            vna.append(t)
        sel_s = wpool.tile([2, 128], F32R, tag="sel")
        nc.sync.dma_start(sel_s[:], sel[:])
        bd_s = wpool.tile([128, E], F32, tag="bd")
        nc.sync.dma_start(bd_s[:], bd_rep[:])
        wd_s = []
        for kk in range(8):
            t = wpool.tile([128, E], BF16, tag=f"wd{kk}")
            nc.sync.dma_start(t[:], wdT[kk * 128 : (kk + 1) * 128, :])
            wd_s.append(t)

        # ---- qe projection: all pairs up-front (2 pairs per PSUM tile) ----
        qe_sb = wpool.tile([128, 8 * SQB], BF16, tag="qe")
        for pp in range(4):
            ps = sc_ps.tile([128, 1024], F32, tag="scps")
            for i in range(2):
                p = 2 * pp + i
                nc.tensor.matmul(
                    ps[:, i * 512 : (i + 1) * 512], wq2_s[:], qin[p][:],
                    start=True, stop=True,
                )
            nc.vector.tensor_scalar(
                qe_sb[:, pp * 1024 : (pp + 1) * 1024], ps[:], cq2_s[:], None,
                op0=ADD,
            )

        U2 = [
            u2p.tile([128, SQB], BF16, tag=f"u2_{p}", name=f"u2_{p}")
            for p in range(8)
        ]
        outT = [None] * 8
        rg2s = [None] * 8
        oproj_ps = [None] * 4

        # ---- software-pipelined: slot p emits scores(p) interleaved with
        # U(p-1) so the in-order PE stream never blocks on exp output; the
        # per-pair normalize runs inline, and output-projection m-blocks 0/1
        # accumulate during the final slot to shorten the tail ----
        Eprev = None  # (EA, EB) of pair p-1
        for p in range(9):
            qe = qe_sb[:, p * SQB : (p + 1) * SQB] if p < 8 else None
            EA, EB = [], []
            if p > 0:
                upsA = u_ps.tile([65, 512], F32, tag="ups", name=f"upsA{p-1}")
                upsB = u_ps.tile([65, 512], F32, tag="ups", name=f"upsB{p-1}")
            if p == 8:
                oproj_ps[0] = sc_ps.tile([128, 1024], F32, tag="scps", name="oproj0")
                oproj_ps[1] = sc_ps.tile([128, 1024], F32, tag="scps", name="oproj1")
            for g in range(8):
                if p < 8:
                    psA = sc_ps.tile([128, 1024], F32, tag="scps")
                    psB = sc_ps.tile([128, 1024], F32, tag="scps")
                    for tt in range(2):
                        t = g * 2 + tt
                        nc.tensor.matmul(
                            psA[:, tt * 512 : (tt + 1) * 512],
                            vt[p][0:64, t * 128 : (t + 1) * 128],
                            qe[0:64, :],
                            start=True, stop=True, tile_position=(0, 0),
                        )
                        nc.tensor.matmul(
                            psB[:, tt * 512 : (tt + 1) * 512],
                            vt[p][64:128, t * 128 : (t + 1) * 128],
                            qe[64:128, :],
                            start=True, stop=True, tile_position=(64, 0),
                        )
                if p > 0:
                    # U(p-1): chunks 2g, 2g+1 for both heads of the pair
                    pEA, pEB = Eprev
                    for h2, ups, EEp in ((0, upsA, pEA), (1, upsB, pEB)):
                        h = 2 * (p - 1) + h2
                        for t in (2 * g, 2 * g + 1):
                            et = EEp[t // 2][:, (t % 2) * 512 : (t % 2 + 1) * 512]
                            nc.tensor.matmul(
                                ups[:], vna[t][:, h * 65 : (h + 1) * 65], et,
                                start=(t == 0), stop=(t == 15),
                            )
                if p == 8 and g < 6:
                    # output projection m-blocks 0/1, contraction chunk kk=g
                    # (kk=6/7 ready only after this slot)
                    for m in range(2):
                        for n in range(2):
                            nc.tensor.matmul(
                                oproj_ps[m][:, n * 512 : (n + 1) * 512],
                                outT[g][:, m * 128 : (m + 1) * 128],
                                wd_s[g][:, n * 512 : (n + 1) * 512],
                                start=(g == 0), stop=False,
                                skip_group_check=True,
                            )
                if p < 8:
                    ea = ep.tile([128, 1024], BF16, tag="E", name=f"ea{p}_{g}")
                    nc.scalar.activation(ea[:], psA[:], EXP, scale=0.125)
                    EA.append(ea)
                    eb = ep.tile([128, 1024], BF16, tag="E", name=f"eb{p}_{g}")
                    if g < ACT_B:
                        nc.scalar.activation(eb[:], psB[:], EXP, scale=0.125)
                    else:
                        nc.vector.tensor_scalar(
                            eb[:].bitcast(I16), psB[:], SCH_A, SCH_B,
                            op0=MULT, op1=ADD,
                        )
                    EB.append(eb)
            if p > 0:
                pm1 = p - 1
                # evacuate U(p-1); row 64 = softmax denominator
                rg2 = osbp.tile([2, SQB], F32, tag="rg2", name=f"rg2_{pm1}")
                rg2s[pm1] = rg2
                for h2, ups in ((0, upsA), (1, upsB)):
                    nc.vector.tensor_copy(
                        U2[pm1][h2 * 64 : (h2 + 1) * 64, :], ups[0:64, :]
                    )
                    # single-partition writes at nonzero offsets are illegal
                    # for compute engines (and DMA can't read PSUM): h0's denom
                    # row lands at partition 0 directly; h1's bounces via DMA
                    if h2 == 0:
                        nc.scalar.copy(rg2[0:1, :], ups[64:65, :])
                    else:
                        rtmp = osbp.tile(
                            [1, SQB], F32, tag="rtmp", name=f"rtmp{pm1}"
                        )
                        nc.vector.tensor_copy(rtmp[:], ups[64:65, :])
                        nc.sync.dma_start(rg2[1:2, :], rtmp[:])
            if p > 1:
                # normalize pair p-2 (deferred a slot: the sel matmul's input
                # chain CAST->DMA->recip gets a full slot of slack, so the
                # in-order PE stream never stalls here)
                pm2 = p - 2
                rr2 = osbp.tile([2, SQB], F32R, tag="rr2", name=f"rr2_{pm2}")
                with nc.allow_low_precision(reason="f32r full fp32 range; f22 mantissa ok for softmax denom"):
                    nc.vector.reciprocal(rr2[:], rg2s[pm2][:])
                rps = sc_ps.tile([128, 1024], F32, tag="scps")
                nc.tensor.matmul(
                    rps[:, 0:512], sel_s[:], rr2[:],
                    start=True, stop=True,
                )
                ot = outp.tile([128, SQB], BF16, tag=f"outT{pm2}")
                nc.vector.tensor_mul(ot[:], U2[pm2][:], rps[:, 0:512])
                outT[pm2] = ot
            Eprev = (EA, EB)

        # ---- normalize pair 7, finish output projection ----
        rr2 = osbp.tile([2, SQB], F32R, tag="rr2", name="rr2_7")
        with nc.allow_low_precision(reason="f32r full fp32 range; f22 mantissa ok for softmax denom"):
            nc.vector.reciprocal(rr2[:], rg2s[7][:])
        rps = sc_ps.tile([128, 1024], F32, tag="scps")
        nc.tensor.matmul(rps[:, 0:512], sel_s[:], rr2[:], start=True, stop=True)
        ot7 = outp.tile([128, SQB], BF16, tag="outT7")
        nc.vector.tensor_mul(ot7[:], U2[7][:], rps[:, 0:512])
        outT[7] = ot7
        for m in range(2):
            for kk in (6, 7):
                for n in range(2):
                    nc.tensor.matmul(
                        oproj_ps[m][:, n * 512 : (n + 1) * 512],
                        outT[kk][:, m * 128 : (m + 1) * 128],
                        wd_s[kk][:, n * 512 : (n + 1) * 512],
                        start=False, stop=(kk == 7), skip_group_check=True,
                    )
            osb = osbp.tile([128, E], F32, tag="osb")
            nc.vector.tensor_add(osb[:], oproj_ps[m][:], bd_s[:])
            nc.sync.dma_start(out[m * 128 : (m + 1) * 128, :], osb[:])
        for m in range(2, 4):
            oproj_ps[m] = sc_ps.tile([128, 1024], F32, tag="scps", name=f"oproj{m}")
            for n in range(2):
                for kk in range(8):
                    nc.tensor.matmul(
                        oproj_ps[m][:, n * 512 : (n + 1) * 512],
                        outT[kk][:, m * 128 : (m + 1) * 128],
                        wd_s[kk][:, n * 512 : (n + 1) * 512],
                        start=(kk == 0), stop=(kk == 7),
                    )
            osb = osbp.tile([128, E], F32, tag="osb")
            nc.vector.tensor_add(osb[:], oproj_ps[m][:], bd_s[:])
            nc.sync.dma_start(out[m * 128 : (m + 1) * 128, :], osb[:])

    nc.compile()
    return nc


def kernel(queries, values, heads, Wv, bv, Wk, bk, Wq, bq, Wd, bd, **_):
    queries = np.asarray(queries, np.float32)
    values = np.asarray(values, np.float32)
    Wv, bv = np.asarray(Wv, np.float64), np.asarray(bv, np.float64)
    Wk = np.asarray(Wk, np.float64)
    Wq, bq = np.asarray(Wq, np.float64), np.asarray(bq, np.float64)
    Wd, bd = np.asarray(Wd, np.float64), np.asarray(bd, np.float64)
    assert int(heads) == H and queries.shape == (B, S, E)

    if "nc" not in _CACHE:
        _CACHE["nc"] = _build_program()
    nc = _CACHE["nc"]

    def blk(A):
        Z = np.zeros_like(A)
        return np.block([[A, Z], [Z, A]]).astype(NP_BF16)

    Wkv = Wk @ Wv
    wq2 = blk(Wq.T @ Wkv)                       # lhsT for qe projection
    cq2 = np.tile(Wkv.T @ bq, 2)[:, None].astype(np.float32)
    sel = np.zeros((2, 128), np.float32)
    sel[0, 0:64] = 1.0
    sel[1, 64:128] = 1.0
    bv_full = np.tile(bv, H)
    bd_rep = np.tile((bd + Wd @ bv_full)[None, :], (128, 1)).astype(np.float32)
    Vblk = np.zeros((E, E))
    for h in range(H):
        Vblk[h * D : (h + 1) * D, h * D : (h + 1) * D] = Wv
    wdT = np.ascontiguousarray((Wd @ Vblk).T).astype(NP_BF16)

    vT_b, vnat_b = [], []
    for b_ in range(B):
        vT_b.append(np.ascontiguousarray(values[b_].T).astype(NP_BF16))
        vn = np.empty((S, H * 65), np.float32)
        vr = values[b_].reshape(S, H, D)
        for h in range(H):
            vn[:, h * 65 : h * 65 + 64] = vr[:, h, :]
            vn[:, h * 65 + 64] = 1.0
        vnat_b.append(vn.astype(NP_BF16))

    common = dict(wq2=wq2, cq2=cq2, sel=sel, wdT=wdT, bd_rep=bd_rep)
    in_maps = []
    for c in range(N_CORES):
        b_, qb = c // 4, c % 4
        in_maps.append(dict(
            qT_in=np.ascontiguousarray(
                queries[b_, qb * SQB : (qb + 1) * SQB, :].T
            ).astype(NP_BF16),
            vT_in=vT_b[b_],
            vnat_in=vnat_b[b_],
            **common,
        ))

    _CACHE["last_in_maps"] = in_maps
    res = run_bass_kernel_spmd(nc, in_maps, list(range(N_CORES)))
    out = np.empty((B, S, E), np.float32)
    for c in range(N_CORES):
        b_, qb = c // 4, c % 4
        out[b_, qb * SQB : (qb + 1) * SQB, :] = res.results[c]["out"]
    return out
